# revision 15
# baseline (speedup 1.0000x reference)
"""Additive (Bahdanau) attention on 8 TRN2 NeuronCores, data-parallel over batch.

Reference math (per batch b):
  qh = queries @ W_q            [Q, H]
  kh = keys @ W_k               [K, H]
  scores[q,k] = sum_h w_v[h] * tanh(qh[q,h] + kh[k,h])
  scores[q,k] = -1e6 where k >= valid_len[b]
  out = softmax_k(scores) @ values

Device strategy (B=16 sharded 2 per core):
  - H=128 lives on the partition axis. khT [H, K] and qhT [H, Q] come from
    PE transposes of the natural loads followed by fp32 projection matmuls,
    evicted to bf16.
  - Per q: DVE tensor_scalar_add broadcasts qhT[:, q] over khT (bf16, split in
    K-halves so adds start before the full khT exists); per q-group one big
    ACT Tanh produces bf16 features (ACT is the roofline: B*Q*K*H/8 elems /
    128 lanes / 1.2GHz ~= 109us/core; the main loop runs tanh back-to-back).
  - Per (q, k-chunk): matmul lhsT=features[H,128] (stationary), rhs=w_v[H,1]
    -> scoresT column [128k, 1] into a one-bank PSUM tile laid out [128, KC*64].
  - Masking is fused into the Exp as a per-partition bias column built from a
    constant iota input and a broadcast valid_len: bias = (k_idx>=len)*-1e6.
    exp(score-1e6) underflows to exactly 0; scores are bounded (~|12|) so no
    max-subtraction is needed.
  - attnT @ [values | ones] accumulates [Q, 257]; the ones column gives the
    softmax denominator; one reciprocal + per-partition scale normalizes.
  - DMA: keys as two big half-DMAs (sync + scalar HWDGE queues), values via a
    single gpsimd SWDGE DMA that casts f32->bf16 in flight.
"""

import numpy as np

import concourse.bass as bass
import concourse.bacc as bacc
import concourse.mybir as mybir
import concourse.tile as tile
from concourse.bass_utils import run_bass_kernel_spmd

B, Q, K, D, H = 16, 64, 1024, 256, 128
NCORES = 8
BL = B // NCORES  # batches per core
KC = K // 128     # k-chunks of 128
DC = D // 128     # d-chunks of 128
QG = 8            # q-group size per Tanh instruction
NEG = -1.0e6

F32 = mybir.dt.float32
BF16 = mybir.dt.bfloat16
I32 = mybir.dt.int32
AF = mybir.ActivationFunctionType
ALU = mybir.AluOpType


def _emit(nc, tc, dram):
    queries, keys, values, vlens, cblobA, cblobB, out = dram
    QSCHED = [4, 4] + [8] * 6 + [4, 4]
    assert sum(QSCHED) == Q
    with (
        tc.tile_pool(name="const", bufs=1) as cpool,
        tc.tile_pool(name="io", bufs=5) as io,
        tc.tile_pool(name="work", bufs=2) as work,
        tc.tile_pool(name="sums", bufs=3) as sums_pool,
        tc.tile_pool(name="feat", bufs=3) as feat_pool,
        tc.tile_pool(name="psT", bufs=3, space=bass.MemorySpace.PSUM) as psT,
        tc.tile_pool(name="psP", bufs=2, space=bass.MemorySpace.PSUM) as psP,
        tc.tile_pool(name="psS", bufs=2, space=bass.MemorySpace.PSUM) as psS,
        tc.tile_pool(name="psO", bufs=1, space=bass.MemorySpace.PSUM) as psO,
    ):
        # small constants first: [ident | ones(row0) | kidx | w_v]
        cbA = cpool.tile([128, 265], F32, tag="cbA")
        nc.sync.dma_start(cbA[:], cblobA[:, :])
        ident_sb = cbA[:, 0:128]
        ones_sb = cbA[0:1, 128:256]
        kidx_sb = cbA[:, 256:264]
        wv_sb = cbA[:, 264:265]
        cbB = cpool.tile([128, 512], F32, tag="cbB")
        wq_sb = cbB[:, 0:256]
        wk_sb = cbB[:, 256:512]
        wq_bf = cpool.tile([128, D], BF16, tag="wqbf")
        wk_bf = cpool.tile([128, D], BF16, tag="wkbf")
        wv_bf = cpool.tile([128, 1], BF16, tag="wvbf")
        vl_i = cpool.tile([1, BL], I32, tag="vli")
        vl_f = cpool.tile([1, BL], F32, tag="vlf")

        for b in range(BL):
            # ---- loads: key chunks first; queries + weights blob mid-stream ----
            knats = []
            for kc in range(KC // 2):
                knat = io.tile([128, D], F32, tag="knat")
                nc.sync.dma_start(knat[:], keys[b, kc * 128 : (kc + 1) * 128, :])
                knats.append(knat)
            qnat = io.tile([Q, D], F32, tag="qnat")
            nc.sync.dma_start(qnat[:], queries[b, :, :])
            if b == 0:
                nc.sync.dma_start(cbB[:], cblobB[:, :])
                nc.sync.dma_start(vl_i[:], vlens[:, :])
            for kc in range(KC // 2, KC):
                knat = io.tile([128, D], F32, tag="knat")
                nc.sync.dma_start(knat[:], keys[b, kc * 128 : (kc + 1) * 128, :])
                knats.append(knat)

            # ---- projections: khT [H, K] (half 0 first), qhT [H, Q] ----
            kTd = [
                [
                    work.tile([128, 512], BF16, tag=f"kTd{dc}{h}", name=f"kTd{dc}{h}")
                    for h in range(2)
                ]
                for dc in range(DC)
            ]
            khT = work.tile([128, K], F32, tag="khT")
            qT_sb = work.tile([128, DC * Q], BF16, tag="qT")
            qhT = work.tile([128, Q], F32, tag="qhT")

            def k_transposes(kc_list):
                for kc in kc_list:
                    for dc in range(DC):
                        tp = psT.tile([128, 128], F32, tag="tp")
                        nc.tensor.transpose(
                            tp[:],
                            knats[kc][:, dc * 128 : (dc + 1) * 128],
                            ident_sb[:, :],
                        )
                        nc.vector.tensor_copy(
                            kTd[dc][kc // 4][:, (kc % 4) * 128 : (kc % 4 + 1) * 128],
                            tp[:],
                        )

            def kh_half(nch):
                kh_ps = psP.tile([128, 512], F32, tag="proj")
                for dc in range(DC):
                    nc.tensor.matmul(
                        kh_ps[:],
                        wk_bf[:, dc * 128 : (dc + 1) * 128],
                        kTd[dc][nch][:],
                        start=(dc == 0),
                        stop=(dc == DC - 1),
                    )
                return nc.vector.tensor_copy(
                    khT[:, nch * 512 : (nch + 1) * 512], kh_ps[:]
                )

            k_transposes(range(0, KC // 2))
            if b == 0:
                nc.vector.tensor_copy(wk_bf[:], wk_sb)
                nc.vector.tensor_copy(wq_bf[:], wq_sb)
            kh_half(0)
            for dc in range(DC):
                tp = psT.tile([128, 128], F32, tag="tp")
                nc.tensor.transpose(
                    tp[:, 0:Q], qnat[:, dc * 128 : (dc + 1) * 128], ident_sb[0:Q, 0:Q]
                )
                nc.vector.tensor_copy(qT_sb[:, dc * Q : (dc + 1) * Q], tp[:, 0:Q])
            qh_ps = psP.tile([128, 512], F32, tag="proj")
            for dc in range(DC):
                nc.tensor.matmul(
                    qh_ps[:, 0:Q],
                    wq_bf[:, dc * 128 : (dc + 1) * 128],
                    qT_sb[:, dc * Q : (dc + 1) * Q],
                    start=(dc == 0),
                    stop=(dc == DC - 1),
                )
            nc.vector.tensor_copy(qhT[:], qh_ps[:, 0:Q])
            k_transposes(range(KC // 2, KC))
            gate_inst = kh_half(1)
            if b == 0:
                nc.vector.tensor_copy(wv_bf[:], wv_sb)
                nc.vector.tensor_copy(vl_f[:], vl_i[:])

            # ---- mask bias column: madd[p, kc] = (p + 128*kc >= len) * -1e6 ----
            ln_ps = psT.tile([128, 128], F32, tag="tp")
            nc.tensor.matmul(
                ln_ps[:, 0:1], ones_sb, vl_f[0:1, b : b + 1], start=True, stop=True
            )
            ln_col = work.tile([128, 1], F32, tag="lncol")
            nc.vector.tensor_copy(ln_col[:], ln_ps[:, 0:1])
            madd = work.tile([128, KC], F32, tag="madd")
            nc.vector.tensor_scalar(
                madd[:], kidx_sb, ln_col[:], NEG, op0=ALU.is_ge, op1=ALU.mult
            )

            # ---- features + scoresT ----
            scT_ps = psS.tile([128, 512], F32, tag="sc")
            q0 = 0
            for g, qg in enumerate(QSCHED):
                sums = sums_pool.tile([128, qg * K], F32, tag="sums")
                for j in range(qg):
                    q = q0 + j
                    nc.vector.tensor_scalar_add(
                        sums[:, j * K : (j + 1) * K], khT[:], qhT[:, q : q + 1]
                    )
                feat = feat_pool.tile([128, qg * K], BF16, tag="feat")
                nc.scalar.activation(feat[:], sums[:], AF.Tanh)
                for j in range(qg):
                    q = q0 + j
                    for kc in range(KC):
                        nc.tensor.matmul(
                            scT_ps[:, kc * 64 + q : kc * 64 + q + 1],
                            feat[:, j * K + kc * 128 : j * K + (kc + 1) * 128],
                            wv_bf[:],
                            start=True,
                            stop=True,
                        )
                q0 += qg

            # ---- masked exp (bias fuses the mask) ----
            pT = work.tile([128, 512], BF16, tag="pT")
            for kc in range(KC):
                nc.scalar.activation(
                    pT[:, kc * 64 : (kc + 1) * 64],
                    scT_ps[:, kc * 64 : (kc + 1) * 64],
                    AF.Exp,
                    bias=madd[:, kc : kc + 1],
                )

            # ---- values (cast to bf16 in the SWDGE DMA), gated off the prologue ----
            vaug = work.tile([128, KC * 260], BF16, tag="vaug")
            # gpsimd runs in order: this tiny copy reads khT, deferring the
            # values DMAs (HBM traffic) until the prologue's loads are done
            nc.gpsimd.tensor_copy(vaug[0:1, 259:260], khT[0:1, 0:1])
            for kc in range(KC):
                nc.gpsimd.dma_start(
                    vaug[:, kc * 260 : kc * 260 + 256],
                    values[b, kc * 128 : (kc + 1) * 128, :],
                )
                nc.gpsimd.memset(vaug[:, kc * 260 + 256 : kc * 260 + 257], 1.0)

            # ---- attnT @ [values | ones], normalize, store ----
            oaug_ps = psO.tile([Q, 257], F32, tag="oa")
            for kc in range(KC):
                nc.tensor.matmul(
                    oaug_ps[:],
                    pT[:, kc * 64 : (kc + 1) * 64],
                    vaug[:, kc * 260 : kc * 260 + 257],
                    start=(kc == 0),
                    stop=(kc == KC - 1),
                )
            recip = work.tile([Q, 1], F32, tag="recip")
            nc.vector.reciprocal(recip[:], oaug_ps[:, 256:257])
            out_sb = work.tile([Q, D], F32, tag="osb")
            nc.vector.tensor_scalar_mul(out_sb[:], oaug_ps[:, 0:256], recip[:])
            nc.sync.dma_start(out[b, :, :], out_sb[:])


def build():
    nc = bacc.Bacc("TRN2", target_bir_lowering=False, debug=False, num_devices=NCORES)
    dram = (
        nc.declare_dram_parameter("queries", [BL, Q, D], F32, isOutput=False),
        nc.declare_dram_parameter("keys", [BL, K, D], F32, isOutput=False),
        nc.declare_dram_parameter("values", [BL, K, D], F32, isOutput=False),
        nc.declare_dram_parameter("valid_lens", [1, BL], I32, isOutput=False),
        nc.declare_dram_parameter("cblobA", [128, 265], F32, isOutput=False),
        nc.declare_dram_parameter("cblobB", [128, 512], F32, isOutput=False),
        nc.declare_dram_parameter("out", [BL, Q, D], F32, isOutput=True),
    )
    with tile.TileContext(nc) as tc:
        _emit(nc, tc, dram)
    nc.compile()
    return nc


_NC = None


def make_in_maps(queries, keys, values, valid_lens, W_q, W_k, w_v):
    queries = np.ascontiguousarray(np.asarray(queries, dtype=np.float32))
    keys = np.ascontiguousarray(np.asarray(keys, dtype=np.float32))
    values = np.ascontiguousarray(np.asarray(values, dtype=np.float32))
    valid_lens = np.asarray(valid_lens, dtype=np.int32)
    W_q = np.asarray(W_q, dtype=np.float32)
    W_k = np.asarray(W_k, dtype=np.float32)
    w_v = np.asarray(w_v, dtype=np.float32).reshape(H)
    cblobA = np.zeros((128, 265), dtype=np.float32)
    cblobA[:, 0:128] = np.eye(128, dtype=np.float32)
    cblobA[0, 128:256] = 1.0
    cblobA[:, 256:264] = (
        np.arange(128, dtype=np.float32)[:, None]
        + 128.0 * np.arange(KC, dtype=np.float32)[None, :]
    )
    cblobA[:, 264] = w_v
    cblobB = np.zeros((128, 512), dtype=np.float32)
    cblobB[:, 0:128] = W_q[0:128, :]
    cblobB[:, 128:256] = W_q[128:256, :]
    cblobB[:, 256:384] = W_k[0:128, :]
    cblobB[:, 384:512] = W_k[128:256, :]
    in_maps = []
    for i in range(NCORES):
        s = slice(i * BL, (i + 1) * BL)
        in_maps.append(
            {
                "queries": np.ascontiguousarray(queries[s]),
                "keys": np.ascontiguousarray(keys[s]),
                "values": np.ascontiguousarray(values[s]),
                "valid_lens": np.ascontiguousarray(valid_lens[s].reshape(1, BL)),
                "cblobA": cblobA,
                "cblobB": cblobB,
            }
        )
    return in_maps


def kernel(queries, keys, values, valid_lens, W_q, W_k, w_v):
    global _NC
    if _NC is None:
        _NC = build()
    in_maps = make_in_maps(queries, keys, values, valid_lens, W_q, W_k, w_v)
    res = run_bass_kernel_spmd(_NC, in_maps, core_ids=list(range(NCORES)))
    return np.concatenate([res.results[i]["out"] for i in range(NCORES)], axis=0)


# revision 16
# speedup vs baseline: 1.1688x; 1.1688x over previous
"""Additive (Bahdanau) attention on 8 TRN2 NeuronCores, data-parallel over batch.

Reference math (per batch b):
  qh = queries @ W_q            [Q, H]
  kh = keys @ W_k               [K, H]
  scores[q,k] = sum_h w_v[h] * tanh(qh[q,h] + kh[k,h])
  scores[q,k] = -1e6 where k >= valid_len[b]
  out = softmax_k(scores) @ values

Device strategy (B=16 sharded 2 per core):
  - H=128 lives on the partition axis. khT [H, K] and qhT [H, Q] come from
    PE transposes of the natural loads followed by fp32 projection matmuls,
    evicted to bf16.
  - Per q: DVE tensor_scalar_add broadcasts qhT[:, q] over khT (bf16, split in
    K-halves so adds start before the full khT exists); per q-group one big
    ACT Tanh produces bf16 features (ACT is the roofline: B*Q*K*H/8 elems /
    128 lanes / 1.2GHz ~= 109us/core; the main loop runs tanh back-to-back).
  - Per (q, k-chunk): matmul lhsT=features[H,128] (stationary), rhs=w_v[H,1]
    -> scoresT column [128k, 1] into a one-bank PSUM tile laid out [128, KC*64].
  - Masking is fused into the Exp as a per-partition bias column built from a
    constant iota input and a broadcast valid_len: bias = (k_idx>=len)*-1e6.
    exp(score-1e6) underflows to exactly 0; scores are bounded (~|12|) so no
    max-subtraction is needed.
  - attnT @ [values | ones] accumulates [Q, 257]; the ones column gives the
    softmax denominator; one reciprocal + per-partition scale normalizes.
  - DMA: keys as two big half-DMAs (sync + scalar HWDGE queues), values via a
    single gpsimd SWDGE DMA that casts f32->bf16 in flight.
"""

import numpy as np

import concourse.bass as bass
import concourse.bacc as bacc
import concourse.mybir as mybir
import concourse.tile as tile
from concourse.bass_utils import run_bass_kernel_spmd

B, Q, K, D, H = 16, 64, 1024, 256, 128
NCORES = 8
BL = B // NCORES  # batches per core
KC = K // 128     # k-chunks of 128
DC = D // 128     # d-chunks of 128
QG = 8            # q-group size per Tanh instruction
NEG = -1.0e6

F32 = mybir.dt.float32
BF16 = mybir.dt.bfloat16
I32 = mybir.dt.int32
AF = mybir.ActivationFunctionType
ALU = mybir.AluOpType


def _emit(nc, tc, dram):
    queries, keys, values, vlens, cblobA, cblobB, out = dram
    QSCHED = [4, 4] + [8] * 6 + [4, 4]
    assert sum(QSCHED) == Q
    with (
        tc.tile_pool(name="const", bufs=1) as cpool,
        tc.tile_pool(name="io", bufs=5) as io,
        tc.tile_pool(name="work", bufs=2) as work,
        tc.tile_pool(name="sums", bufs=3) as sums_pool,
        tc.tile_pool(name="feat", bufs=3) as feat_pool,
        tc.tile_pool(name="psT", bufs=3, space=bass.MemorySpace.PSUM) as psT,
        tc.tile_pool(name="psP", bufs=2, space=bass.MemorySpace.PSUM) as psP,
        tc.tile_pool(name="psS", bufs=2, space=bass.MemorySpace.PSUM) as psS,
        tc.tile_pool(name="psO", bufs=1, space=bass.MemorySpace.PSUM) as psO,
    ):
        cbA = cpool.tile([128, 265], F32, tag="cbA")
        nc.sync.dma_start(cbA[:], cblobA[:, :])
        ident_sb = cbA[:, 0:128]
        ones_sb = cbA[0:1, 128:256]
        kidx_sb = cbA[:, 256:264]
        wv_sb = cbA[:, 264:265]
        cbB = cpool.tile([128, 512], F32, tag="cbB")
        nc.sync.dma_start(cbB[:], cblobB[:, :])
        wq_sb = cbB[:, 0:256]
        wk_sb = cbB[:, 256:512]
        wq_bf = cpool.tile([128, D], BF16, tag="wqbf")
        nc.vector.tensor_copy(wq_bf[:], wq_sb)
        wk_bf = cpool.tile([128, D], BF16, tag="wkbf")
        nc.vector.tensor_copy(wk_bf[:], wk_sb)
        wv_bf = cpool.tile([128, 1], BF16, tag="wvbf")
        nc.vector.tensor_copy(wv_bf[:], wv_sb)
        vl_i = cpool.tile([1, BL], I32, tag="vli")
        nc.sync.dma_start(vl_i[:], vlens[:, :])
        vl_f = cpool.tile([1, BL], F32, tag="vlf")
        nc.vector.tensor_copy(vl_f[:], vl_i[:])

        for b in range(BL):
            qnat = io.tile([Q, D], F32, tag="qnat")
            nc.sync.dma_start(qnat[:], queries[b, :, :])
            knats = []
            for kc in range(KC):
                knat = io.tile([128, D], F32, tag="knat")
                nc.sync.dma_start(knat[:], keys[b, kc * 128 : (kc + 1) * 128, :])
                knats.append(knat)

            # ---- projections: khT [H, K] (half 0 first), qhT [H, Q] ----
            kTd = [
                [
                    work.tile([128, 512], BF16, tag=f"kTd{dc}{h}", name=f"kTd{dc}{h}")
                    for h in range(2)
                ]
                for dc in range(DC)
            ]
            khT = work.tile([128, K], F32, tag="khT")
            qT_sb = work.tile([128, DC * Q], BF16, tag="qT")
            qhT = work.tile([128, Q], F32, tag="qhT")

            def k_transposes(kc_list):
                for kc in kc_list:
                    for dc in range(DC):
                        tp = psT.tile([128, 128], F32, tag="tp")
                        nc.tensor.transpose(
                            tp[:],
                            knats[kc][:, dc * 128 : (dc + 1) * 128],
                            ident_sb[:, :],
                        )
                        nc.vector.tensor_copy(
                            kTd[dc][kc // 4][:, (kc % 4) * 128 : (kc % 4 + 1) * 128],
                            tp[:],
                        )

            def kh_half(nch):
                kh_ps = psP.tile([128, 512], F32, tag="proj")
                for dc in range(DC):
                    nc.tensor.matmul(
                        kh_ps[:],
                        wk_bf[:, dc * 128 : (dc + 1) * 128],
                        kTd[dc][nch][:],
                        start=(dc == 0),
                        stop=(dc == DC - 1),
                    )
                nc.vector.tensor_copy(khT[:, nch * 512 : (nch + 1) * 512], kh_ps[:])

            k_transposes(range(0, KC // 2))
            kh_half(0)
            for dc in range(DC):
                tp = psT.tile([128, 128], F32, tag="tp")
                nc.tensor.transpose(
                    tp[:, 0:Q], qnat[:, dc * 128 : (dc + 1) * 128], ident_sb[0:Q, 0:Q]
                )
                nc.vector.tensor_copy(qT_sb[:, dc * Q : (dc + 1) * Q], tp[:, 0:Q])
            qh_ps = psP.tile([128, 512], F32, tag="proj")
            for dc in range(DC):
                nc.tensor.matmul(
                    qh_ps[:, 0:Q],
                    wq_bf[:, dc * 128 : (dc + 1) * 128],
                    qT_sb[:, dc * Q : (dc + 1) * Q],
                    start=(dc == 0),
                    stop=(dc == DC - 1),
                )
            nc.vector.tensor_copy(qhT[:], qh_ps[:, 0:Q])
            k_transposes(range(KC // 2, KC))
            kh_half(1)

            # ---- mask bias column: madd[p, kc] = (p + 128*kc >= len) * -1e6 ----
            ln_ps = psT.tile([128, 128], F32, tag="tp")
            nc.tensor.matmul(
                ln_ps[:, 0:1], ones_sb, vl_f[0:1, b : b + 1], start=True, stop=True
            )
            ln_col = work.tile([128, 1], F32, tag="lncol")
            nc.vector.tensor_copy(ln_col[:], ln_ps[:, 0:1])
            madd = work.tile([128, KC], F32, tag="madd")
            nc.vector.tensor_scalar(
                madd[:], kidx_sb, ln_col[:], NEG, op0=ALU.is_ge, op1=ALU.mult
            )

            # ---- features + scoresT ----
            scT_ps = psS.tile([128, 512], F32, tag="sc")
            q0 = 0
            for g, qg in enumerate(QSCHED):
                sums = sums_pool.tile([128, qg * K], F32, tag="sums")
                for j in range(qg):
                    q = q0 + j
                    nc.vector.tensor_scalar_add(
                        sums[:, j * K : (j + 1) * K], khT[:], qhT[:, q : q + 1]
                    )
                feat = feat_pool.tile([128, qg * K], BF16, tag="feat")
                nc.scalar.activation(feat[:], sums[:], AF.Tanh)
                for j in range(qg):
                    q = q0 + j
                    for kc in range(KC):
                        nc.tensor.matmul(
                            scT_ps[:, kc * 64 + q : kc * 64 + q + 1],
                            feat[:, j * K + kc * 128 : j * K + (kc + 1) * 128],
                            wv_bf[:],
                            start=True,
                            stop=True,
                        )
                q0 += qg

            # ---- masked exp (bias fuses the mask) ----
            pT = work.tile([128, 512], BF16, tag="pT")
            for kc in range(KC):
                nc.scalar.activation(
                    pT[:, kc * 64 : (kc + 1) * 64],
                    scT_ps[:, kc * 64 : (kc + 1) * 64],
                    AF.Exp,
                    bias=madd[:, kc : kc + 1],
                )

            # ---- values (cast to bf16 in the SWDGE DMA) with ones columns ----
            vaug = work.tile([128, KC * 260], BF16, tag="vaug")
            for kc in range(KC):
                nc.gpsimd.dma_start(
                    vaug[:, kc * 260 : kc * 260 + 256],
                    values[b, kc * 128 : (kc + 1) * 128, :],
                )
                nc.gpsimd.memset(vaug[:, kc * 260 + 256 : kc * 260 + 257], 1.0)

            # ---- attnT @ [values | ones], normalize, store ----
            oaug_ps = psO.tile([Q, 257], F32, tag="oa")
            for kc in range(KC):
                nc.tensor.matmul(
                    oaug_ps[:],
                    pT[:, kc * 64 : (kc + 1) * 64],
                    vaug[:, kc * 260 : kc * 260 + 257],
                    start=(kc == 0),
                    stop=(kc == KC - 1),
                )
            recip = work.tile([Q, 1], F32, tag="recip")
            nc.vector.reciprocal(recip[:], oaug_ps[:, 256:257])
            out_sb = work.tile([Q, D], F32, tag="osb")
            nc.vector.tensor_scalar_mul(out_sb[:], oaug_ps[:, 0:256], recip[:])
            nc.sync.dma_start(out[b, :, :], out_sb[:])


def build():
    nc = bacc.Bacc("TRN2", target_bir_lowering=False, debug=False, num_devices=NCORES)
    dram = (
        nc.declare_dram_parameter("queries", [BL, Q, D], F32, isOutput=False),
        nc.declare_dram_parameter("keys", [BL, K, D], F32, isOutput=False),
        nc.declare_dram_parameter("values", [BL, K, D], F32, isOutput=False),
        nc.declare_dram_parameter("valid_lens", [1, BL], I32, isOutput=False),
        nc.declare_dram_parameter("cblobA", [128, 265], F32, isOutput=False),
        nc.declare_dram_parameter("cblobB", [128, 512], F32, isOutput=False),
        nc.declare_dram_parameter("out", [BL, Q, D], F32, isOutput=True),
    )
    with tile.TileContext(nc) as tc:
        _emit(nc, tc, dram)
    nc.compile()
    return nc


_NC = None


def make_in_maps(queries, keys, values, valid_lens, W_q, W_k, w_v):
    queries = np.ascontiguousarray(np.asarray(queries, dtype=np.float32))
    keys = np.ascontiguousarray(np.asarray(keys, dtype=np.float32))
    values = np.ascontiguousarray(np.asarray(values, dtype=np.float32))
    valid_lens = np.asarray(valid_lens, dtype=np.int32)
    W_q = np.asarray(W_q, dtype=np.float32)
    W_k = np.asarray(W_k, dtype=np.float32)
    w_v = np.asarray(w_v, dtype=np.float32).reshape(H)
    cblobA = np.zeros((128, 265), dtype=np.float32)
    cblobA[:, 0:128] = np.eye(128, dtype=np.float32)
    cblobA[0, 128:256] = 1.0
    cblobA[:, 256:264] = (
        np.arange(128, dtype=np.float32)[:, None]
        + 128.0 * np.arange(KC, dtype=np.float32)[None, :]
    )
    cblobA[:, 264] = w_v
    cblobB = np.zeros((128, 512), dtype=np.float32)
    cblobB[:, 0:128] = W_q[0:128, :]
    cblobB[:, 128:256] = W_q[128:256, :]
    cblobB[:, 256:384] = W_k[0:128, :]
    cblobB[:, 384:512] = W_k[128:256, :]
    in_maps = []
    for i in range(NCORES):
        s = slice(i * BL, (i + 1) * BL)
        in_maps.append(
            {
                "queries": np.ascontiguousarray(queries[s]),
                "keys": np.ascontiguousarray(keys[s]),
                "values": np.ascontiguousarray(values[s]),
                "valid_lens": np.ascontiguousarray(valid_lens[s].reshape(1, BL)),
                "cblobA": cblobA,
                "cblobB": cblobB,
            }
        )
    return in_maps


def kernel(queries, keys, values, valid_lens, W_q, W_k, w_v):
    global _NC
    if _NC is None:
        _NC = build()
    in_maps = make_in_maps(queries, keys, values, valid_lens, W_q, W_k, w_v)
    res = run_bass_kernel_spmd(_NC, in_maps, core_ids=list(range(NCORES)))
    return np.concatenate([res.results[i]["out"] for i in range(NCORES)], axis=0)
